# revision 32
# baseline (speedup 1.0000x reference)
"""Bilinear interaction layer (nn_BilinearInteractionLayer) on 8 TRN2 cores.

out[b, p*64+e] = (sum_d x[b, i_p, d] * W[p, d, e]) * x[b, j_p, e]
  with (i_p, j_p) the p-th pair of triu_indices(32, k=1), B=2048, D=64, P=496.

Sharding: data-parallel over batch (8 x 256 rows); W replicated on every core.
kernel(**inputs) takes the FULL inputs, shards on host, runs one SPMD Bass
program on cores 0..7 via run_bass_kernel_spmd, and concatenates the per-core
[256, 31744] outputs back to [2048, 31744] (float32, matching the reference).

Per-core kernel. Matmul form out[b,e] = xT_i.T @ W[p] puts the result in
natural [batch, e] layout, so the vj elementwise multiply and the output DMA
need no further transposes and every output DMA row is a contiguous DRAM run:
  - x natural [256, 2048] in SBUF (the vj operand of the multiply)
  - xt host-pretransposed [128, 4096]: rows 0:64 hold even features as
    [d, batch], rows 64:128 odd features. Stationary (lhsT) tiles [64, 128].
  - W host-packed [128, 16384]: rows 0:64 = the 256 even-i pairs' [d, e]
    blocks, rows 64:128 = the 240 odd-i pairs (zero-padded). The moving (rhs)
    operand for one matmul is 8 consecutive pairs = [64, 512].
  - K=64 matmuls run on PE row halves 0:64 / 64:128 (tile_position derives
    from the operand base partition), so even-i and odd-i matmuls overlap on
    the array.
  - Matmul outputs land packed in multi-bank PSUM tiles; the DVE multiplies
    each PSUM block by the matching contiguous slice of x (j runs
    consecutively within an i-block) straight into an SBUF staging tile;
    one output DMA per (b_tile, adjacent-i-block-pair) writes [128 rows x
    up to 15.6KB] contiguous chunks.
"""

import numpy as np

F = 32
D = 64
NPAIR = F * (F - 1) // 2  # 496
B = 2048
NCORES = 8
BS = B // NCORES  # 256
PD = NPAIR * D  # 31744

_EVEN_I = list(range(0, F - 1, 2))  # 0..30
_ODD_I = list(range(1, F - 1, 2))  # 1..29 (31 has no pairs)


def _off(i):
    # start pair-index of the i-block in natural triu order
    return (F - 1) * i - i * (i - 1) // 2


def _cum(idx_list):
    c, out = 0, {}
    for i in idx_list:
        out[i] = c
        c += (F - 1) - i
    return out, c


_CUM_EVEN, _N_EVEN = _cum(_EVEN_I)  # 256
_CUM_ODD, _N_ODD = _cum(_ODD_I)  # 240

_NC_CACHE = {}

# Kernel variant. Base dtype: "float32" (bit-exact fp32, PE streams 4 cyc/col)
# or "f32r" (FP32R single-pass, 1 cyc/col, tf32-class rounding, ~2.2e-4
# scale-relative absmax err vs fp32 reference). Suffixes: "_bigdve2" batches
# matmul outputs into 2-bank PSUM tiles so the vj elementwise multiply runs as
# ~76 large DVE ops instead of 140 (DVE is the #2 engine); "_notr" transposes
# x on the PE instead of shipping a host-pretransposed copy.
# Measured (8 cores, per-iteration HW time, same-session comparison):
#   float32 114-297us | f32r 91-123us | f32r_bigdve2 91.5-119.6us (best)
DTYPE = "f32r_bigdve2"


def _build_nc(dtype_name="float32", repeat=1):
    import concourse.mybir as mybir
    import concourse.tile as tile
    from concourse import bacc

    key = (dtype_name, repeat)
    if key in _NC_CACHE:
        return _NC_CACHE[key]

    f32 = mybir.dt.float32
    # float32r: PE streams 1 col/cycle (vs 4 for plain fp32) at tf32-class
    # precision (~1.6e-4 rel err measured); float32 is bit-exact vs reference.
    base, _, suffix = dtype_name.partition("_")
    mm_dt = mybir.dt.float32r if base == "f32r" else f32
    on_chip_tr = "notr" in suffix  # transpose x on the PE instead of host xt
    big_dve = "bigdve" in suffix  # multi-bank PSUM tiles + fewer, larger DVE ops
    ps_banks = 2 if ("bigdve2" in suffix or on_chip_tr) else 4
    ps_bufs = (8 // ps_banks) if big_dve else (5 if on_chip_tr else 6)
    if big_dve and on_chip_tr:
        ps_bufs = 3  # 3*2 banks + 2 transpose banks = 8
    nc = bacc.Bacc("TRN2", target_bir_lowering=False, debug=False)

    x_d = nc.dram_tensor("x", [BS, F * D], f32, kind="ExternalInput")
    xt_d = ident_d = None
    if on_chip_tr:
        ident_d = nc.dram_tensor("ident", [128, 128], f32, kind="ExternalInput")
    else:
        xt_d = nc.dram_tensor("xt", [128, 16 * BS], f32, kind="ExternalInput")
    w_d = nc.dram_tensor("w", [128, _N_EVEN * D], f32, kind="ExternalInput")
    y_d = nc.dram_tensor("y", [BS, PD], f32, kind="ExternalOutput")

    with tile.TileContext(nc) as tc:
        import contextlib

        with (
            tc.tile_pool(name="const", bufs=1) as const,
            tc.tile_pool(name="xp", bufs=2) as xpool,
            tc.tile_pool(name="ps", bufs=ps_bufs, space="PSUM") as pspool,
            tc.tile_pool(name="ps2", bufs=2, space="PSUM") as pspool2,
            tc.tile_pool(name="op", bufs=3) as opool,
            (tc.For_i(0, repeat, 1) if repeat > 1 else contextlib.nullcontext()),
        ):
            w_buf = const.tile([128, _N_EVEN * D], mm_dt, tag="w")
            xt_buf = const.tile([128, 16 * BS], mm_dt, tag="xt")
            ident = None
            if on_chip_tr:
                # DMA the identity (host np.eye) rather than memset+affine
                # -select: those ops reject the f32r dtype in walrus codegen.
                ident = const.tile([128, 128], mm_dt, tag="ident")
                nc.sync.dma_start(ident[:, :], ident_d[:, :].bitcast(mm_dt))
            else:
                nc.sync.dma_start(xt_buf[:, :], xt_d[:, :].bitcast(mm_dt))
            wcols = _N_EVEN * D
            for q in range(4):
                c0, c1 = q * wcols // 4, (q + 1) * wcols // 4
                nc.sync.dma_start(w_buf[:, c0:c1], w_d[:, c0:c1].bitcast(mm_dt))

            for t in range(BS // 128):
                x_tile = xpool.tile([128, F * D], mm_dt, tag="x")
                nc.sync.dma_start(x_tile[:, :], x_d[t * 128 : (t + 1) * 128, :].bitcast(mm_dt))

                if on_chip_tr:
                    # x_tile cols f*128..(f+1)*128 cover features (2f, 2f+1);
                    # PE transpose -> PSUM [128 d-pair, 128 b]: partitions 0:64
                    # = feature 2f, 64:128 = feature 2f+1 — exactly xt layout.
                    for f in range(16):
                        tp = pspool2.tile([128, 128], mm_dt, tag="tp")
                        nc.tensor.transpose(
                            tp[:, :],
                            x_tile[:, f * 128 : (f + 1) * 128],
                            ident[:, :],
                        )
                        nc.vector.tensor_copy(
                            xt_buf[:, f * BS + t * 128 : f * BS + t * 128 + 128],
                            tp[:, :],
                        )

                for k in range(16):
                    ilo, ihi = 2 * k, 2 * k + 1
                    np_lo = (F - 1) - ilo
                    np_hi = (F - 1) - ihi if ihi < F - 1 else 0
                    total = np_lo + np_hi
                    stg = opool.tile([128, total * D], f32, tag="stg")

                    glo = [(s, min(8, np_lo - s)) for s in range(0, np_lo, 8)]
                    ghi = [(s, min(8, np_hi - s)) for s in range(0, np_hi, 8)]

                    if big_dve:
                        # One PSUM tile (up to ps_banks banks) per half-round;
                        # each group MM targets a bank-aligned slice; one DVE
                        # multiply per psum tile (chunks of ps_banks*8 pairs).
                        halves = [("lo", ilo, 0, 0, np_lo, glo)]
                        if np_hi:
                            halves.append(("hi", ihi, np_lo * D, 64, np_hi, ghi))
                        chunk_pairs = ps_banks * 8
                        ps_tiles = {}  # (half, chunk_idx) -> tile
                        dve_jobs = []
                        for half, i, base, r0, npair, groups in halves:
                            for c0p in range(0, npair, chunk_pairs):
                                cp = min(chunk_pairs, npair - c0p)
                                pst = pspool.tile(
                                    [128, ps_banks * 512], f32, tag="ps", name="psbig"
                                )
                                ps_tiles[(half, c0p // chunk_pairs)] = pst
                                dve_jobs.append((half, i, base, c0p, cp, pst))
                        # interleave lo/hi MMs for PE row-half overlap
                        seq = []
                        for idx in range(max(len(glo), len(ghi))):
                            for half_info in halves:
                                if idx < len(half_info[5]):
                                    seq.append((half_info, half_info[5][idx]))
                        for (half, i, base, r0, npair, groups), (s, gs) in seq:
                            n = gs * D
                            gidx = (_CUM_EVEN[i] if half == "lo" else _CUM_ODD[i]) + s
                            fi = i // 2
                            lhsT = xt_buf[
                                r0 : r0 + 64,
                                fi * BS + t * 128 : fi * BS + t * 128 + 128,
                            ]
                            rhs = w_buf[r0 : r0 + 64, gidx * D : gidx * D + n]
                            pst = ps_tiles[(half, s // chunk_pairs)]
                            so = (s % chunk_pairs) * D
                            nc.tensor.matmul(
                                pst[:, so : so + n],
                                lhsT,
                                rhs,
                                start=True,
                                stop=True,
                            )
                        for half, i, base, c0p, cp, pst in dve_jobs:
                            nc.vector.tensor_mul(
                                out=stg[:, base + c0p * D : base + (c0p + cp) * D],
                                in0=pst[:, : cp * D],
                                in1=x_tile[
                                    :, (i + 1 + c0p) * D : (i + 1 + c0p + cp) * D
                                ].bitcast(f32),
                            )
                    else:
                        seq = []
                        for idx in range(max(len(glo), len(ghi))):
                            if idx < len(glo):
                                seq.append(("lo", glo[idx]))
                            if idx < len(ghi):
                                seq.append(("hi", ghi[idx]))

                        for half, (s, gs) in seq:
                            n = gs * D
                            if half == "lo":
                                i, base, r0 = ilo, 0, 0
                                gidx = _CUM_EVEN[i] + s
                            else:
                                i, base, r0 = ihi, np_lo * D, 64
                                gidx = _CUM_ODD[i] + s
                            fi = i // 2
                            j0 = i + 1 + s
                            ps = pspool.tile([128, 512], f32, tag="ps")
                            lhsT = xt_buf[
                                r0 : r0 + 64,
                                fi * BS + t * 128 : fi * BS + t * 128 + 128,
                            ]
                            rhs = w_buf[r0 : r0 + 64, gidx * D : gidx * D + n]
                            nc.tensor.matmul(
                                ps[:, :n], lhsT, rhs, start=True, stop=True
                            )
                            nc.vector.tensor_mul(
                                out=stg[:, base + s * D : base + s * D + n],
                                in0=ps[:, :n],
                                in1=x_tile[:, j0 * D : j0 * D + n].bitcast(f32),
                            )

                    c0 = _off(ilo) * D
                    nc.sync.dma_start(
                        y_d[t * 128 : (t + 1) * 128, c0 : c0 + total * D], stg[:, :]
                    )

    nc.finalize()
    _NC_CACHE[key] = nc
    return nc


def _prep_inputs(inputs, W, host_xt=True):
    inputs = np.ascontiguousarray(np.asarray(inputs, dtype=np.float32))
    W = np.ascontiguousarray(np.asarray(W, dtype=np.float32))

    even_p = [p for p, i in enumerate(_pair_i()) if i % 2 == 0]
    odd_p = [p for p, i in enumerate(_pair_i()) if i % 2 == 1]
    w_packed = np.zeros((128, _N_EVEN * D), dtype=np.float32)
    w_packed[0:64, :] = W[even_p].transpose(1, 0, 2).reshape(64, _N_EVEN * D)
    w_packed[64:128, : _N_ODD * D] = (
        W[odd_p].transpose(1, 0, 2).reshape(64, _N_ODD * D)
    )

    in_maps = []
    for c in range(NCORES):
        xs = inputs[c * BS : (c + 1) * BS]  # [256, 32, 64]
        x_flat = np.ascontiguousarray(xs.reshape(BS, F * D))
        m = {"x": x_flat, "w": w_packed}
        if not host_xt:
            m["ident"] = np.eye(128, dtype=np.float32)
        if host_xt:
            xtt = xs.transpose(2, 1, 0)  # [64, 32, 256]
            xt = np.empty((128, 16 * BS), dtype=np.float32)
            xt[0:64, :] = np.ascontiguousarray(xtt[:, 0::2, :]).reshape(64, 16 * BS)
            xt[64:128, :] = np.ascontiguousarray(xtt[:, 1::2, :]).reshape(64, 16 * BS)
            m["xt"] = xt
        in_maps.append(m)
    return in_maps


_PAIR_I = None


def _pair_i():
    global _PAIR_I
    if _PAIR_I is None:
        _PAIR_I = [i for i in range(F) for _ in range(i + 1, F)]
    return _PAIR_I


def _run(inputs, W, trace=False, trace_cores=None, dtype_name=None):
    from concourse.bass_utils import run_bass_kernel_spmd

    dn = dtype_name or DTYPE
    nc = _build_nc(dn)
    in_maps = _prep_inputs(inputs, W, host_xt="_notr" not in dn)
    res = run_bass_kernel_spmd(
        nc,
        in_maps,
        core_ids=list(range(NCORES)),
        trace=trace,
        trace_cores=trace_cores,
    )
    out = np.concatenate([res.results[c]["y"] for c in range(NCORES)], axis=0)
    return out, res


def kernel(inputs, W):
    out, _ = _run(inputs, W, trace=False)
    return out
